# revision 8
# baseline (speedup 1.0000x reference)
"""Trainium2 Bass kernel for nn_MultiHeadAttention_58548994179754.

Sharding: 8 cores = 2 batches x 4 head-groups (4 heads of 64 dims each per core).
Per core:
  - xqT [256,2048] = (Wq_g @ q[b]^T)*SCALE + bq_g*SCALE   (transposed layout; bias via an
    augmented ones-row in the contraction)
  - xkT [256,2048] likewise for the K rows of Wkv
  - xv  [2048,256] in natural layout plus a ones column per head (softmax denominator)
  - per head: logitsT[k,q] matmul; u = exp(logitsT + maskf[k]) (mask = per-partition ACT
    bias); attn_unnormT[d,q] + denom row = xv_aug^T @ u accumulated over k-tiles;
    normalize by 1/(denom+1)  [add-one softmax with shift m=0, mathematically identical
    to the reference's max-shift]
  - out_partial [2048,1024] = attn_core @ Wo_g^T (row-slice of the O-projection)
Host: out[b] = sum of the 4 cores' partials + bo.

All matmuls run as float32r (single-pass fp32, 4x the fp32 PE rate; inputs are bitcast
views of the f32 tiles).  Even/odd heads of a pair sit on PE row-groups 0-1/2-3 so their
K=64 QK matmuls overlap in the array.
"""

import os
import sys

import numpy as np

sys.path.insert(0, "/opt/trn_rl_repo")

B, N, E = 2, 2048, 1024
H, D = 16, 64
HPC = 4  # heads per core
DPC = HPC * D  # 256 output dims per core
SCALE = float(E) ** -0.5
KC = 9  # contraction chunks: 8x128 data + 1 chunk (bias/ones row + zero pad)
CPAD = KC * 128  # 1152
NT = N // 128  # 16 seq tiles
MASK_NEG = np.float32(-1e30)

_CACHED = {}


def build_bass(repeat=1):
    import concourse.bass as bass
    import concourse.mybir as mybir
    import concourse.tile as tile
    from concourse import bacc

    f32 = mybir.dt.float32
    f32r = mybir.dt.float32r
    Exp = mybir.ActivationFunctionType.Exp

    nc = bacc.Bacc("TRN2", target_bir_lowering=False)

    qT = nc.dram_tensor("qT", (KC, 128, N), f32r, kind="ExternalInput")
    kvT = nc.dram_tensor("kvT", (KC, 128, N), f32r, kind="ExternalInput")
    wq = nc.dram_tensor("wq", (KC, 128, DPC), f32r, kind="ExternalInput")
    wk = nc.dram_tensor("wk", (KC, 128, DPC), f32r, kind="ExternalInput")
    wv = nc.dram_tensor("wv", (KC, 128, DPC), f32r, kind="ExternalInput")
    wo = nc.dram_tensor("wo", (2, 128, E), f32r, kind="ExternalInput")
    mk = nc.dram_tensor("mk", (128, NT), f32, kind="ExternalInput")
    ones1 = nc.dram_tensor("ones1", (128, NT, HPC, 1), f32r, kind="ExternalInput")
    outp = nc.dram_tensor("outp", (N, E), f32, kind="ExternalOutput")

    def mm(out, lhsT, rhs, **kw):
        nc.tensor.matmul(out, lhsT, rhs, **kw)

    with tile.TileContext(nc) as tc:
        with (
            tc.tile_pool(name="const", bufs=1) as const,
            tc.tile_pool(name="io", bufs=3) as io_pool,
            tc.tile_pool(name="ups", bufs=3) as upool,
            tc.tile_pool(name="rps", bufs=2) as rpool,
            tc.tile_pool(name="osb", bufs=2) as osb,
            tc.tile_pool(name="ps", bufs=4, space="PSUM") as ps,
        ):
            wq_sb = const.tile([128, KC, DPC], f32r, name="wq_sb")
            wk_sb = const.tile([128, KC, DPC], f32r, name="wk_sb")
            wv_sb = const.tile([128, KC, DPC], f32r, name="wv_sb")
            wo_sb = const.tile([128, 2, E], f32r, name="wo_sb")
            mk_sb = const.tile([128, NT], f32, name="mk_sb")
            xqT = const.tile([128, 2, N], f32r, name="xqT")
            xkT = const.tile([128, 2, N], f32r, name="xkT")
            xv = const.tile([128, NT, HPC, D + 1], f32r, name="xv")
            attn = const.tile([128, 2, N], f32r, name="attn")
            warm = const.tile([1, 8], f32, name="warm")

            for kc in range(KC):
                nc.sync.dma_start(wq_sb[:, kc], wq[kc])
                nc.sync.dma_start(wk_sb[:, kc], wk[kc])
                nc.sync.dma_start(wv_sb[:, kc], wv[kc])
            nc.sync.dma_start(wo_sb[:, 0], wo[0])
            nc.sync.dma_start(wo_sb[:, 1], wo[1])
            nc.sync.dma_start(mk_sb[:], mk[:])
            # ones columns for the softmax-denominator rows
            nc.sync.dma_start(xv[:, :, :, D : D + 1], ones1[:])
            # warm up the ACT exp table early so the table load overlaps phase A
            nc.vector.memset(warm[:], 0.0)
            nc.scalar.activation(warm[:], warm[:], Exp)

            def body(_iv=None):
                # ---------------- Phase A: projections ----------------
                # Q: two halves of n; two 128-row m-tiles per half
                for half in range(2):
                    hsl = slice(half * 1024, (half + 1) * 1024)
                    qps = [
                        ps.tile([128, 1024], f32, tag="ps", name=f"qps{half}_{m}")
                        for m in range(2)
                    ]
                    for kc in range(KC):
                        qt = io_pool.tile([128, 1024], f32r, tag="io", name="qt")
                        nc.sync.dma_start(qt[:], qT[kc, :, hsl])
                        for m in range(2):
                            for g in range(2):
                                mm(
                                    qps[m][:, g * 512 : (g + 1) * 512],
                                    wq_sb[:, kc, m * 128 : (m + 1) * 128],
                                    qt[:, g * 512 : (g + 1) * 512],
                                    start=(kc == 0),
                                    stop=(kc == KC - 1),
                                )
                    for m in range(2):
                        nc.vector.tensor_copy(xqT[:, m, hsl], qps[m][:])
                # K and V share a fully-resident kvT half
                for half in range(2):
                    hsl = slice(half * 1024, (half + 1) * 1024)
                    kv_sb = io_pool.tile(
                        [128, KC, 1024], f32r, tag="kvres", name=f"kv_sb{half}", bufs=1
                    )
                    for kc in range(KC):
                        nc.sync.dma_start(kv_sb[:, kc], kvT[kc, :, hsl])
                    kps = [
                        ps.tile([128, 1024], f32, tag="ps", name=f"kps{half}_{m}")
                        for m in range(2)
                    ]
                    for kc in range(KC):
                        for m in range(2):
                            for g in range(2):
                                mm(
                                    kps[m][:, g * 512 : (g + 1) * 512],
                                    wk_sb[:, kc, m * 128 : (m + 1) * 128],
                                    kv_sb[:, kc, g * 512 : (g + 1) * 512],
                                    start=(kc == 0),
                                    stop=(kc == KC - 1),
                                )
                    for m in range(2):
                        nc.vector.tensor_copy(xkT[:, m, hsl], kps[m][:])
                    for nt in range(8):
                        gnt = half * 8 + nt
                        vp = ps.tile([128, 256], f32, tag="ps", name=f"vp{half}_{nt}")
                        for kc in range(KC):
                            mm(
                                vp[:],
                                kv_sb[:, kc, nt * 128 : (nt + 1) * 128],
                                wv_sb[:, kc, :],
                                start=(kc == 0),
                                stop=(kc == KC - 1),
                            )
                        for h in range(HPC):
                            nc.vector.tensor_copy(
                                xv[:, gnt, h, 0:D], vp[:, h * 64 : (h + 1) * 64]
                            )

                # ---------------- Phase B: attention ----------------
                for hp in range(2):  # head pair (chunk of xqT/xkT partitions)
                    for qb in range(2):  # 1024-wide query blocks
                        avs = [
                            ps.tile([65, 1024], f32, tag="ps", name=f"av{hp}{qb}_{i}")
                            for i in range(2)
                        ]
                        for kt in range(NT):
                            us = []
                            for h2 in range(2):
                                po = 64 * h2
                                qk = ps.tile(
                                    [128, 1024], f32, tag="ps", name=f"qk{h2}"
                                )
                                for s in range(2):
                                    mm(
                                        qk[:, s * 512 : (s + 1) * 512],
                                        xkT[po : po + 64, hp, kt * 128 : (kt + 1) * 128],
                                        xqT[
                                            po : po + 64,
                                            hp,
                                            qb * 1024 + s * 512 : qb * 1024 + (s + 1) * 512,
                                        ],
                                        start=True,
                                        stop=True,
                                    )
                                u = upool.tile([128, 1024], f32r, tag="u", name=f"u{h2}")
                                nc.scalar.activation(
                                    u[:], qk[:], Exp, bias=mk_sb[:, kt : kt + 1], scale=1.0
                                )
                                us.append(u)
                            for h2 in range(2):
                                h = 2 * hp + h2
                                for s in range(2):
                                    mm(
                                        avs[h2][:, s * 512 : (s + 1) * 512],
                                        xv[:, kt, h, :],
                                        us[h2][:, s * 512 : (s + 1) * 512],
                                        start=(kt == 0),
                                        stop=(kt == NT - 1),
                                    )
                        for h2 in range(2):
                            av = avs[h2]
                            po = 64 * h2
                            rs = rpool.tile([1, 1024], f32, tag="rs", name="rs")
                            nc.vector.tensor_scalar_add(rs[:], av[64:65, :], 1.0)
                            nc.vector.reciprocal(rs[:], rs[:])
                            rb = rpool.tile([64, 1024], f32, tag="rb", name="rb")
                            nc.gpsimd.partition_broadcast(rb[:], rs[0:1, :])
                            nc.vector.tensor_mul(
                                attn[po : po + 64, hp, qb * 1024 : (qb + 1) * 1024],
                                av[0:64, :],
                                rb[:],
                            )

                # ---------------- Phase C: output projection ----------------
                for nt in range(NT):
                    op = ps.tile([128, 1024], f32, tag="ps", name="op")
                    for c in range(2):
                        for s in range(2):
                            mm(
                                op[:, s * 512 : (s + 1) * 512],
                                attn[:, c, nt * 128 : (nt + 1) * 128],
                                wo_sb[:, c, s * 512 : (s + 1) * 512],
                                start=(c == 0),
                                stop=(c == 1),
                            )
                    ot = osb.tile([128, 1024], f32, tag="ot", name="ot")
                    nc.vector.tensor_copy(ot[:], op[:])
                    nc.sync.dma_start(outp[nt * 128 : (nt + 1) * 128, :], ot[:])

            if repeat == 1:
                body()
            else:
                with tc.For_i(0, repeat, 1) as _i:
                    body(_i)

    nc.compile()
    return nc


def make_in_maps(q, kv, mask, Wq, bq, Wkv, bkv, Wo, bo):
    q = np.asarray(q, dtype=np.float32)
    kv = np.asarray(kv, dtype=np.float32)
    mask = np.asarray(mask)
    Wq = np.asarray(Wq, dtype=np.float32)
    bq = np.asarray(bq, dtype=np.float32)
    Wkv = np.asarray(Wkv, dtype=np.float32)
    bkv = np.asarray(bkv, dtype=np.float32)
    Wo = np.asarray(Wo, dtype=np.float32)

    Wk, Wv = Wkv[:E], Wkv[E:]
    bk, bv = bkv[:E], bkv[E:]

    qTa = {}
    kvTa = {}
    mks = {}
    for b in range(B):
        t = np.zeros((CPAD, N), np.float32)
        t[:E] = q[b].T
        t[E] = 1.0
        qTa[b] = t.reshape(KC, 128, N)
        t = np.zeros((CPAD, N), np.float32)
        t[:E] = kv[b].T
        t[E] = 1.0
        kvTa[b] = t.reshape(KC, 128, N)
        mf = np.where(mask[b] == 0, MASK_NEG, mask[b].astype(np.float32))
        mks[b] = np.ascontiguousarray(mf.reshape(NT, 128).T)

    in_maps = []
    for c in range(8):
        b, g = divmod(c, 4)
        hs = slice(DPC * g, DPC * (g + 1))

        wqa = np.zeros((CPAD, DPC), np.float32)
        wqa[:E] = Wq[hs].T * SCALE
        wqa[E] = bq[hs] * SCALE
        wka = np.zeros((CPAD, DPC), np.float32)
        wka[:E] = Wk[hs].T
        wka[E] = bk[hs]
        wva = np.zeros((CPAD, DPC), np.float32)
        wva[:E] = Wv[hs].T
        wva[E] = bv[hs]
        woT = np.ascontiguousarray(Wo[:, hs].T)  # [256, 1024]

        in_maps.append(
            {
                "qT": qTa[b],
                "kvT": kvTa[b],
                "wq": wqa.reshape(KC, 128, DPC),
                "wk": wka.reshape(KC, 128, DPC),
                "wv": wva.reshape(KC, 128, DPC),
                "wo": woT.reshape(2, 128, E),
                "mk": mks[b],
                "ones1": np.ones((128, NT, HPC, 1), np.float32),
            }
        )
    return in_maps


def kernel(q, kv, mask, Wq, bq, Wkv, bkv, Wo, bo, _repeat=1):
    from concourse.bass_utils import run_bass_kernel_spmd

    key = f"nc_{_repeat}"
    if key not in _CACHED:
        _CACHED[key] = build_bass(repeat=_repeat)
    nc = _CACHED[key]

    in_maps = make_in_maps(q, kv, mask, Wq, bq, Wkv, bkv, Wo, bo)
    res = run_bass_kernel_spmd(nc, in_maps, core_ids=list(range(8)))
    _CACHED["last_result"] = res

    bo = np.asarray(bo, dtype=np.float32)
    outs = [res.results[c]["outp"] for c in range(8)]
    out = np.stack(
        [
            outs[0] + outs[1] + outs[2] + outs[3],
            outs[4] + outs[5] + outs[6] + outs[7],
        ]
    )
    out += bo[None, None, :]
    return out.astype(np.float32)
